# revision 12
# baseline (speedup 1.0000x reference)
"""Trainium2 Bass kernel for nn_Loss_46883863003176.

loss = sum((predictions - targets)**2) / (2d+1) / batch_size
with predictions/targets of shape (4096, 2047, 2) float32.

Strategy (data-parallel over 8 NeuronCores, hybrid fp8/bf16 + PE Gram):
  - Host casts ~53% of each core's [128, 16376]-flattened shard to fp8
    e4m3 and the rest to bf16 (loss tolerance is 2e-2; quantization
    contributes ~4e-4 relative error). The split balances the HBM
    stream (~390 GB/s effective on the sync HWDGE ring) against DVE
    subtract throughput: fp8 halves bytes but tensor_sub runs 1x on
    fp8 vs 2x on bf16.
  - DVE subtracts each tile into a contiguous bf16 d-buffer.
  - TensorE squares+reduces everything via Gram accumulation: for each
    128-column chunk C of d, matmul(G += C.T @ C) into one PSUM bank
    (measured 0.83 ns/unit sustained - LDWEIGHTS pipelines with
    MATMUL). diag(G) holds per-column-mod-128 sums of squares; the
    host takes trace(G). ACT copies G to SBUF at the end; sync DMAs
    it out (the scalar HWDGE ring is ~10x slower - never store there).
  - Output per core: the [128, 128] f32 Gram matrix. Host sums traces
    in float64 and divides by (2d+1)*batch_size.

Measured (bench_ops.py): DVE sub bf16 (f/2+151)/0.96ns, fp8
(f+151)/0.96ns; engines can downclock ~20% while DMA streams; PE Gram
0.83ns/unit; tensor_tensor_reduce crashes HW (avoid); GpSimd tensor
ops block DVE (shared SBUF port, avoid).
"""

import sys

if "/opt/trn_rl_repo" not in sys.path:
    sys.path.insert(0, "/opt/trn_rl_repo")

import numpy as np

B = 4096          # batch
S = 2047          # 2*d+1
C = 2             # coords
N_CORES = 8
ROWS = B // N_CORES          # 512 batch rows per core
PER_CORE = ROWS * S * C      # 2,096,128 elements
P = 128                      # SBUF partitions
FREE = PER_CORE // P         # 16376 elements per partition

# (dtype, size) in load order. fp8 first so DVE builds backlog while
# the cheap-to-subtract bf16 tiles stream later; small tail tile.
TILES = [("f8", 512), ("f8", 1792), ("f8", 1792), ("f8", 1792),
         ("f8", 1792), ("f8", 1792), ("f8", 1280), ("bf", 1280),
         ("bf", 1280), ("bf", 1024), ("bf", 1024), ("bf", 760),
         ("bf", 256)]
NT = len(TILES)
assert sum(f for _, f in TILES) == FREE
NF = sum(f for d, f in TILES if d == "f8")   # 8704 fp8 units
NB = FREE - NF                               # 7672 bf16 units

_CACHE = {}


def _build():
    import concourse.bass as bass  # noqa: F401
    from concourse import bacc, mybir

    nc = bacc.Bacc(
        "TRN2", debug=False, target_bir_lowering=False, num_devices=N_CORES
    )
    f32 = mybir.dt.float32
    bf16 = mybir.dt.bfloat16
    fp8 = mybir.dt.float8e4

    # host interleaves [p-tile | t-tile] per tile: one DMA per tile
    # instead of two (the sync queue's ~0.65us per-DMA issue cost was
    # the stream bottleneck at 26 DMAs).
    x8_ap = nc.dram_tensor("x8", [P, 2 * NF], fp8, kind="ExternalInput").ap()
    xb_ap = nc.dram_tensor("xb", [P, 2 * NB], bf16, kind="ExternalInput").ap()
    g_ap = nc.dram_tensor("g", [P, P], f32, kind="ExternalOutput").ap()

    xin = []
    for j, (dt, f) in enumerate(TILES):
        sb_dt = fp8 if dt == "f8" else bf16
        xin.append(nc.alloc_sbuf_tensor(f"xin{j}", [P, 2 * f], sb_dt).ap())
    d_sb = nc.alloc_sbuf_tensor("dsb", [P, FREE], bf16).ap()
    g_sb = nc.alloc_sbuf_tensor("gsb", [P, P], f32).ap()
    g_psum = nc.alloc_psum_tensor("gpsum", [P, P], f32).ap()

    pair_sems = [nc.alloc_semaphore(f"pair{j}") for j in range(NT)]
    v_sem = nc.alloc_semaphore("v_sem")       # completed subtracts
    pe_sem = nc.alloc_semaphore("pe_sem")     # PE accumulation done
    cp_sem = nc.alloc_semaphore("cp_sem")     # G copied to SBUF
    store_sem = nc.alloc_semaphore("store_sem")

    o8 = ob = od = 0
    src_off, d_off = [], []
    for dt, f in TILES:
        if dt == "f8":
            src_off.append(o8)
            o8 += f
        else:
            src_off.append(ob)
            ob += f
        d_off.append(od)
        od += f

    # no_gpsimd_drain: no SWDGE DMAs are issued, so Pool's expensive
    # exit dge_drain is unnecessary; sem-only exit barrier trims the
    # postamble.
    with nc.Block(no_gpsimd_drain=True) as block:

        @block.sync
        def _(sync):
            for j, (dt, f) in enumerate(TILES):
                o = 2 * src_off[j]
                x_ap = x8_ap if dt == "f8" else xb_ap
                sync.dma_start(xin[j][:], x_ap[:, o : o + 2 * f]).then_inc(
                    pair_sems[j], 16
                )
            # Store from the sync ring (fast); scalar's ring is ~32 GB/s.
            sync.wait_ge(cp_sem, 1)
            sync.dma_start(g_ap[:], g_sb[:]).then_inc(store_sem, 16)

        @block.vector
        def _(vector):
            for j, (dt, f) in enumerate(TILES):
                o = d_off[j]
                vector.wait_ge(pair_sems[j], 16)
                vector.tensor_sub(
                    d_sb[:, o : o + f], xin[j][:, :f], xin[j][:, f:]
                ).then_inc(v_sem, 1)

        @block.tensor
        def _(tensor):
            first = True
            mm = None
            for j, (dt, f) in enumerate(TILES):
                o = d_off[j]
                tensor.wait_ge(v_sem, j + 1)
                for c in range(0, f, P):
                    w = min(P, f - c)
                    sl = d_sb[:, o + c : o + c + w]
                    last = (j == NT - 1) and (c + w >= f)
                    mm = tensor.matmul(
                        g_psum[:w, :w] if w < P else g_psum[:],
                        sl,
                        sl,
                        start=first,
                        stop=last,
                        skip_group_check=True,
                    )
                    first = False
            mm.then_inc(pe_sem, 1)

        @block.scalar
        def _(scalar):
            scalar.wait_ge(pe_sem, 1)
            scalar.activation(
                g_sb[:], g_psum[:], mybir.ActivationFunctionType.Copy
            ).then_inc(cp_sem, 1)

    nc.compile()
    return nc


def _get_nc():
    if "nc" not in _CACHE:
        _CACHE["nc"] = _build()
    return _CACHE["nc"]


def _tile_offs():
    o8 = ob = 0
    offs = []
    for dt, f in TILES:
        offs.append(o8 if dt == "f8" else ob)
        if dt == "f8":
            o8 += f
        else:
            ob += f
    return offs


def _shard2(pred, targ):
    """(B, S, C) f32 pair -> per-core interleaved [p-tile | t-tile]
    tensors: x8 [128, 2*NF] fp8 (first NF flat units) and xb
    [128, 2*NB] bf16 (rest)."""
    import ml_dtypes

    pf = np.ascontiguousarray(pred).reshape(N_CORES, P, FREE)
    tf = np.ascontiguousarray(targ).reshape(N_CORES, P, FREE)
    x8 = np.empty((N_CORES, P, 2 * NF), dtype=ml_dtypes.float8_e4m3)
    xb = np.empty((N_CORES, P, 2 * NB), dtype=ml_dtypes.bfloat16)
    offs = _tile_offs()
    for j, (dt, f) in enumerate(TILES):
        o = offs[j]
        if dt == "f8":
            src_lo = o
            x8[:, :, 2 * o : 2 * o + f] = pf[:, :, src_lo : src_lo + f]
            x8[:, :, 2 * o + f : 2 * o + 2 * f] = tf[:, :, src_lo : src_lo + f]
        else:
            src_lo = NF + o
            xb[:, :, 2 * o : 2 * o + f] = pf[:, :, src_lo : src_lo + f]
            xb[:, :, 2 * o + f : 2 * o + 2 * f] = tf[:, :, src_lo : src_lo + f]
    return x8, xb


def _run(in_maps, **kwargs):
    from concourse.bass_utils import run_bass_kernel_spmd

    return run_bass_kernel_spmd(_get_nc(), in_maps, list(range(N_CORES)), **kwargs)


def kernel(predictions, targets, d, batch_size, **_ignored):
    d_i = int(np.asarray(d))
    bs = int(np.asarray(batch_size))
    s_i = 2 * d_i + 1

    pred = np.asarray(predictions, dtype=np.float32)
    targ = np.asarray(targets, dtype=np.float32)

    if bs != B or s_i != S or pred.shape != (B, S, C):
        # Shape fell outside the compiled layout; numpy fallback keeps the
        # contract correct for any input.
        diff = (pred[:bs, :s_i, :C] - targ[:bs, :s_i, :C]).astype(np.float64)
        return np.float32((diff * diff).sum() / s_i / bs)

    x8, xb = _shard2(pred, targ)
    in_maps = [{"x8": x8[c], "xb": xb[c]} for c in range(N_CORES)]
    res = _run(in_maps).results

    total = 0.0
    for r in res:
        total += float(np.trace(r["g"].astype(np.float64)))
    return np.float32(total / s_i / bs)


# revision 13
# speedup vs baseline: 1.0093x; 1.0093x over previous
"""Trainium2 Bass kernel for nn_Loss_46883863003176.

loss = sum((predictions - targets)**2) / (2d+1) / batch_size
with predictions/targets of shape (4096, 2047, 2) float32.

Strategy (data-parallel over 8 NeuronCores, hybrid fp8/bf16 + PE Gram):
  - Host casts ~53% of each core's [128, 16376]-flattened shard to fp8
    e4m3 and the rest to bf16 (loss tolerance is 2e-2; quantization
    contributes ~4e-4 relative error). The split balances the HBM
    stream (~390 GB/s effective on the sync HWDGE ring) against DVE
    subtract throughput: fp8 halves bytes but tensor_sub runs 1x on
    fp8 vs 2x on bf16.
  - DVE subtracts each tile into a contiguous bf16 d-buffer.
  - TensorE squares+reduces everything via Gram accumulation: for each
    128-column chunk C of d, matmul(G += C.T @ C) into one PSUM bank
    (measured 0.83 ns/unit sustained - LDWEIGHTS pipelines with
    MATMUL). diag(G) holds per-column-mod-128 sums of squares; the
    host takes trace(G). ACT copies G to SBUF at the end; sync DMAs
    it out (the scalar HWDGE ring is ~10x slower - never store there).
  - Output per core: the [128, 128] f32 Gram matrix. Host sums traces
    in float64 and divides by (2d+1)*batch_size.

Measured (bench_ops.py): DVE sub bf16 (f/2+151)/0.96ns, fp8
(f+151)/0.96ns; engines can downclock ~20% while DMA streams; PE Gram
0.83ns/unit; tensor_tensor_reduce crashes HW (avoid); GpSimd tensor
ops block DVE (shared SBUF port, avoid).
"""

import sys

if "/opt/trn_rl_repo" not in sys.path:
    sys.path.insert(0, "/opt/trn_rl_repo")

import numpy as np

B = 4096          # batch
S = 2047          # 2*d+1
C = 2             # coords
N_CORES = 8
ROWS = B // N_CORES          # 512 batch rows per core
PER_CORE = ROWS * S * C      # 2,096,128 elements
P = 128                      # SBUF partitions
FREE = PER_CORE // P         # 16376 elements per partition

# (dtype, size) in load order. fp8 first so DVE builds backlog while
# the cheap-to-subtract bf16 tiles stream later; small tail tile.
TILES = [("f8", 512), ("f8", 896), ("f8", 896), ("f8", 1792),
         ("f8", 1792), ("f8", 1792), ("f8", 1792), ("f8", 1280),
         ("bf", 1280), ("bf", 1280), ("bf", 1024), ("bf", 1024),
         ("bf", 760), ("bf", 256)]
# Tile squared by ACT (Square+accum into g column P) instead of PE:
# drains the PE chunk backlog at the tail while ACT is otherwise idle.
ACT_TILE = 11
NT = len(TILES)
assert sum(f for _, f in TILES) == FREE
NF = sum(f for d, f in TILES if d == "f8")   # 8704 fp8 units
NB = FREE - NF                               # 7672 bf16 units

_CACHE = {}


def _build():
    import concourse.bass as bass  # noqa: F401
    from concourse import bacc, mybir

    nc = bacc.Bacc(
        "TRN2", debug=False, target_bir_lowering=False, num_devices=N_CORES
    )
    f32 = mybir.dt.float32
    bf16 = mybir.dt.bfloat16
    fp8 = mybir.dt.float8e4

    # host interleaves [p-tile | t-tile] per tile: one DMA per tile
    # instead of two (the sync queue's ~0.65us per-DMA issue cost was
    # the stream bottleneck at 26 DMAs).
    x8_ap = nc.dram_tensor("x8", [P, 2 * NF], fp8, kind="ExternalInput").ap()
    xb_ap = nc.dram_tensor("xb", [P, 2 * NB], bf16, kind="ExternalInput").ap()
    # column P holds ACT's accumulated squares for tile ACT_TILE
    g_ap = nc.dram_tensor("g", [P, P + 1], f32, kind="ExternalOutput").ap()

    xin = []
    for j, (dt, f) in enumerate(TILES):
        sb_dt = fp8 if dt == "f8" else bf16
        xin.append(nc.alloc_sbuf_tensor(f"xin{j}", [P, 2 * f], sb_dt).ap())
    d_sb = nc.alloc_sbuf_tensor("dsb", [P, FREE], bf16).ap()
    g_sb = nc.alloc_sbuf_tensor("gsb", [P, P + 1], f32).ap()
    g_psum = nc.alloc_psum_tensor("gpsum", [P, P], f32).ap()
    sqdump = nc.alloc_sbuf_tensor(
        "sqdump", [P, TILES[ACT_TILE][1]], bf16
    ).ap()

    pair_sems = [nc.alloc_semaphore(f"pair{j}") for j in range(NT)]
    v_sem = nc.alloc_semaphore("v_sem")       # completed subtracts
    pe_sem = nc.alloc_semaphore("pe_sem")     # PE accumulation done
    cp_sem = nc.alloc_semaphore("cp_sem")     # G copied to SBUF
    store_sem = nc.alloc_semaphore("store_sem")

    o8 = ob = od = 0
    src_off, d_off = [], []
    for dt, f in TILES:
        if dt == "f8":
            src_off.append(o8)
            o8 += f
        else:
            src_off.append(ob)
            ob += f
        d_off.append(od)
        od += f

    # no_gpsimd_drain: no SWDGE DMAs are issued, so Pool's expensive
    # exit dge_drain is unnecessary; sem-only exit barrier trims the
    # postamble.
    with nc.Block(no_gpsimd_drain=True) as block:

        @block.sync
        def _(sync):
            for j, (dt, f) in enumerate(TILES):
                o = 2 * src_off[j]
                x_ap = x8_ap if dt == "f8" else xb_ap
                sync.dma_start(xin[j][:], x_ap[:, o : o + 2 * f]).then_inc(
                    pair_sems[j], 16
                )
            # Store from the sync ring (fast); scalar's ring is ~32 GB/s.
            sync.wait_ge(cp_sem, 1)
            sync.dma_start(g_ap[:], g_sb[:]).then_inc(store_sem, 16)

        @block.vector
        def _(vector):
            for j, (dt, f) in enumerate(TILES):
                o = d_off[j]
                vector.wait_ge(pair_sems[j], 16)
                vector.tensor_sub(
                    d_sb[:, o : o + f], xin[j][:, :f], xin[j][:, f:]
                ).then_inc(v_sem, 1)

        @block.tensor
        def _(tensor):
            first = True
            mm = None
            for j, (dt, f) in enumerate(TILES):
                if j == ACT_TILE:
                    continue
                o = d_off[j]
                tensor.wait_ge(v_sem, j + 1)
                for c in range(0, f, P):
                    w = min(P, f - c)
                    sl = d_sb[:, o + c : o + c + w]
                    last = (j == NT - 1) and (c + w >= f)
                    mm = tensor.matmul(
                        g_psum[:w, :w] if w < P else g_psum[:],
                        sl,
                        sl,
                        start=first,
                        stop=last,
                        skip_group_check=True,
                    )
                    first = False
            mm.then_inc(pe_sem, 1)

        @block.scalar
        def _(scalar):
            fa = TILES[ACT_TILE][1]
            oa = d_off[ACT_TILE]
            scalar.wait_ge(v_sem, ACT_TILE + 1)
            scalar.activation(
                sqdump[:],
                d_sb[:, oa : oa + fa],
                mybir.ActivationFunctionType.Square,
                accum_out=g_sb[:, P : P + 1],
            )
            scalar.wait_ge(pe_sem, 1)
            scalar.activation(
                g_sb[:, :P], g_psum[:], mybir.ActivationFunctionType.Copy
            ).then_inc(cp_sem, 1)

    nc.compile()
    return nc


def _get_nc():
    if "nc" not in _CACHE:
        _CACHE["nc"] = _build()
    return _CACHE["nc"]


def _tile_offs():
    o8 = ob = 0
    offs = []
    for dt, f in TILES:
        offs.append(o8 if dt == "f8" else ob)
        if dt == "f8":
            o8 += f
        else:
            ob += f
    return offs


def _shard2(pred, targ):
    """(B, S, C) f32 pair -> per-core interleaved [p-tile | t-tile]
    tensors: x8 [128, 2*NF] fp8 (first NF flat units) and xb
    [128, 2*NB] bf16 (rest)."""
    import ml_dtypes

    pf = np.ascontiguousarray(pred).reshape(N_CORES, P, FREE)
    tf = np.ascontiguousarray(targ).reshape(N_CORES, P, FREE)
    x8 = np.empty((N_CORES, P, 2 * NF), dtype=ml_dtypes.float8_e4m3)
    xb = np.empty((N_CORES, P, 2 * NB), dtype=ml_dtypes.bfloat16)
    offs = _tile_offs()
    for j, (dt, f) in enumerate(TILES):
        o = offs[j]
        if dt == "f8":
            src_lo = o
            x8[:, :, 2 * o : 2 * o + f] = pf[:, :, src_lo : src_lo + f]
            x8[:, :, 2 * o + f : 2 * o + 2 * f] = tf[:, :, src_lo : src_lo + f]
        else:
            src_lo = NF + o
            xb[:, :, 2 * o : 2 * o + f] = pf[:, :, src_lo : src_lo + f]
            xb[:, :, 2 * o + f : 2 * o + 2 * f] = tf[:, :, src_lo : src_lo + f]
    return x8, xb


def _run(in_maps, **kwargs):
    from concourse.bass_utils import run_bass_kernel_spmd

    return run_bass_kernel_spmd(_get_nc(), in_maps, list(range(N_CORES)), **kwargs)


def kernel(predictions, targets, d, batch_size, **_ignored):
    d_i = int(np.asarray(d))
    bs = int(np.asarray(batch_size))
    s_i = 2 * d_i + 1

    pred = np.asarray(predictions, dtype=np.float32)
    targ = np.asarray(targets, dtype=np.float32)

    if bs != B or s_i != S or pred.shape != (B, S, C):
        # Shape fell outside the compiled layout; numpy fallback keeps the
        # contract correct for any input.
        diff = (pred[:bs, :s_i, :C] - targ[:bs, :s_i, :C]).astype(np.float64)
        return np.float32((diff * diff).sum() / s_i / bs)

    x8, xb = _shard2(pred, targ)
    in_maps = [{"x8": x8[c], "xb": xb[c]} for c in range(N_CORES)]
    res = _run(in_maps).results

    total = 0.0
    for r in res:
        g = r["g"].astype(np.float64)
        total += float(np.trace(g[:, :P])) + float(g[:, P].sum())
    return np.float32(total / s_i / bs)
